# revision 1
# baseline (speedup 1.0000x reference)
"""Trainium2 Bass kernel for CKANConv2d (KAN conv: SiLU base + B-spline path).

Math: for each output pixel p and output channel co:
  out[co,p] = sum_{c,kh,kw} silu(x[c,p+k]) * Wb[co,(c,kh,kw)]
            + sum_{c,kh,kw,g} B_g(x[c,p+k]) * Ws[co,(c,kh,kw),g]
with B_g the order-3 uniform B-spline bases over knots {-2.2 + 0.4j}.

Key identity used on-chip (t = 2.5*x, center c_g = g - 3.5):
  v = |2.5 x - c_g|; m = min(v,2)-2; n = min(m+1,0)
  B_g(x) = (4 n^3 - m^3) / 6
The 1/6 is folded into the spline weights; the bases are computed per
*input* pixel (not per unfolded patch, 9x less work) and the 3x3
convolution is done as an implicit GEMM over 9 shifted windows with
contraction (c,g) packed 128 rows at a time.

Sharding: data-parallel over batch, 1 image per NeuronCore (8 cores).
"""
import numpy as np
import ml_dtypes

B, CIN, H, W = 8, 64, 56, 56
COUT, K = 128, 3
HO = WO = 54
NOUT = HO * WO  # 2916
NTAP = K * K  # 9
NKT = 4  # spline K-tiles per tap: 128 rows = 64c x 2g, 4 tiles cover g=0..7
NGRP = 6  # output row groups of 9 rows each
GROW = 9  # output rows per group
NFREE = GROW * WO  # 486 <= 512 (one PSUM bank)

_CACHE = {}


def _chunk_rows(j):
    """Input-row range (start, end) whose bases are computed in chunk j.
    Group r's matmuls read input rows [9r, 9r+10]; chunk j<=r covers them."""
    if j == 0:
        return 0, 11
    return 9 * j + 2, 9 * j + 11


def _patch_tile_tail_drain():
    """walrus in this env rejects the Tile tail Drain when it carries >1
    sync waits; split them into a chain of single-wait Drains."""
    import concourse.tile as tile
    from concourse.vector_clock import ScopedClock

    if getattr(tile.TileContext, "_drain_patched", False):
        return

    def _patched(self, tick_clock, wait_clock):
        drain_inst = self.nc.sync.drain()
        wait_clock.add_sem_waits(
            drain_inst.ins, ScopedClock({None: tick_clock.global_clock})
        )
        si = drain_inst.ins.sync_info
        waits = list(si.on_wait) if si is not None else []
        if len(waits) > 1:
            si.on_wait = waits[:1]
            handles = {h.num: h for h in self.sems.allocated().values()}
            for w in waits[1:]:
                extra = self.nc.sync.drain()
                extra.wait_op(handles[w.id], w.wait_value, "sem-ge")
        self.nc.all_engine_barrier()
        assert self.sems is not None
        popped = self.nc._tile_sem_poison_stack.pop()
        assert popped is self._sem_poison
        self.nc.clear_and_free_semaphores(list(self.sems.allocated().values()))
        self.nc.all_engine_barrier()

    tile.TileContext._drain_and_barrier = _patched
    tile.TileContext._drain_patched = True


def _split_excess_waits(nc, max_waits=1):
    """This walrus build encodes at most one sync-wait per instruction.
    Move extra waits onto same-engine NoOps inserted just before."""
    import bass_rust
    from concourse import mybir

    for f in nc.m.functions:
        for bb in f.blocks:
            new = []
            for ins in bb.instructions:
                si = ins.sync_info
                if si is not None and len(si.on_wait) > max_waits:
                    waits = list(si.on_wait)
                    for w in waits[: len(waits) - max_waits]:
                        nop = mybir.InstNoOp(
                            name=nc.get_next_instruction_name(), ins=[], outs=[]
                        )
                        nop.engine = ins.engine
                        h = bass_rust.SemaphoreHandle(name=w.ant_name, num=w.id)
                        bass_rust.wait_op(nop, h, w.wait_value, "sem-ge", False)
                        nc.register_instruction(nop, overwrite=True)
                        new.append(nop)
                    si.on_wait = waits[len(waits) - max_waits :]
                new.append(ins)
            bb.instructions = new


CFG = {"nchunks": 6, "vchunks": 3, "ew16": True, "sq_engine": "act", "evict_engine": "dve", "pair_base": False, "swdgeq": 4, "wdma": "gpsimd", "do_ew": True, "do_base": True, "do_spline": True, "sbufs": 2, "pbufs": 6, "shift_base": True}


def _chunks_for(nchunks):
    """Partition input rows 0..55 into nchunks contiguous chunks such that
    matmul group r (needs input rows 9r..9r+10) only depends on chunks
    emitted at or before group r. Returns list of (r0, r1, first_group)
    where first_group is the earliest group index that must wait for it."""
    # groups per chunk: split 6 groups as evenly as possible
    import math
    gper = [len(x) for x in np.array_split(np.arange(6), nchunks)]
    out = []
    g0 = 0
    r_prev = 0
    for k, ng in enumerate(gper):
        glast = g0 + ng - 1
        r1 = min(9 * glast + 11, 56)
        out.append((r_prev, r1, g0))
        r_prev = r1
        g0 += ng
    return out


def _build(cfg=None):
    key = ("nc", tuple(sorted((cfg or CFG).items())))
    if key in _CACHE:
        return _CACHE[key]
    cfg = dict(CFG, **(cfg or {}))
    _patch_tile_tail_drain()
    import concourse.bass as bass
    import concourse.tile as tile
    from concourse import mybir

    f32 = mybir.dt.float32
    bf16 = mybir.dt.bfloat16
    ew = bf16 if cfg["ew16"] else f32
    Alu = mybir.AluOpType
    Act = mybir.ActivationFunctionType

    nc = bass.Bass("TRN2", num_swdge_queues=cfg["swdgeq"])
    x_d = nc.dram_tensor("x", [CIN, H, W], f32, kind="ExternalInput").ap()
    wspl_d = nc.dram_tensor(
        "wspl", [128, NTAP * NKT, 128], bf16, kind="ExternalInput"
    ).ap()
    # base weights arranged in concurrent row-tile pairs: pair j holds tap 2j
    # on partitions 0:64 and tap 2j+1 on 64:128 (tap 8 alone in pair 4).
    wbase_d = nc.dram_tensor("wbase2", [128, 5, 128], bf16, kind="ExternalInput").ap()
    bneg_d = nc.dram_tensor("betaneg", [128, NKT], f32, kind="ExternalInput").ap()
    wbflat_d = nc.dram_tensor("wbflat", [CIN, NTAP, 128], bf16, kind="ExternalInput").ap()
    y_d = nc.dram_tensor("y", [128, HO, WO], f32, kind="ExternalOutput").ap()

    WB = W * NKT  # 224: 4 k-tile column blocks side by side

    nrep = cfg.get("replicate", 1)
    with tile.TileContext(nc) as tc:
        with (
            tc.tile_pool(name="consts", bufs=1) as cpool,
            tc.tile_pool(name="scratch", bufs=cfg.get("sbufs", 3)) as spool,
            tc.tile_pool(name="psum", bufs=cfg.get("pbufs", 3), space="PSUM") as ppool,
        ):
          for _rep in range(nrep):
              bneg = cpool.tile([128, NKT], f32, tag="bneg")
              nc.sync.dma_start(bneg[:], bneg_d)
              # x first, row-chunked so chunk-0 elementwise starts immediately
              RR = 12  # first x-DMA rows (chunk 0 + 1 shifted-silu row)
              x2 = cpool.tile([128, H, W], f32, tag="x2")
              nc.sync.dma_start(x2[0:CIN, 0:RR, :], x_d[:, 0:RR, :])
              nc.sync.dma_start(x2[CIN:128, 0:RR, :], x_d[:, 0:RR, :])
              wdma = nc.gpsimd.dma_start if cfg["wdma"] == "gpsimd" else nc.sync.dma_start
              wbase = None
              wbase_flat = None
              if cfg["shift_base"]:
                  wbase = cpool.tile([128, 5, 128], bf16, tag="wbase")
                  wdma(wbase[:], wbase_d)
              else:
                  wbase_flat = cpool.tile([CIN, NTAP, 128], bf16, tag="wbflat")
                  wdma(wbase_flat[:], wbflat_d)
              wspl = cpool.tile([128, NTAP * NKT, 128], bf16, tag="wspl")
              wdma(wspl[:, 0:NTAP, :], wspl_d[:, 0:NTAP, :])

              RS = 11  # silu/chunk-0 row boundary
              silu2 = cpool.tile([128, H, W], bf16, tag="silu2")
              siluB = None
              if cfg["shift_base"]:
                  siluB = cpool.tile([128, H, W], bf16, tag="siluB")

              def emit_silu(r0, r1):
                  # lower = silu(x) on ACT; shifted upper halves via SBUF-SBUF
                  # DMA partition-offset copies (DMA engines are idle here)
                  if cfg["shift_base"]:
                      nc.scalar.activation(
                          silu2[0:CIN, r0:r1, :], x2[0:CIN, r0:r1, :], Act.Silu
                      )
                      # S_A upper: shift (0,+1)
                      nc.sync.dma_start(
                          silu2[CIN:128, r0:r1, 0 : W - 1],
                          silu2[0:CIN, r0:r1, 1:W],
                      )
                  else:
                      nc.scalar.activation(
                          silu2[:, r0:r1, :], x2[:, r0:r1, :], Act.Silu
                      )

              def emit_siluB(r0, r1):
                  # S_B: lower = silu(x); upper = silu(x) shifted (+1,-2)
                  r1u = min(r1, H - 1)
                  nc.sync.dma_start(
                      siluB[CIN:128, r0:r1u, 2:W],
                      silu2[0:CIN, r0 + 1 : r1u + 1, 0 : W - 2],
                  )
                  nc.sync.dma_start(
                      siluB[0:CIN, r0:r1, :], silu2[0:CIN, r0:r1, :]
                  )

              emit_silu(0, RS)
              nc.sync.dma_start(x2[0:CIN, RR:H, :], x_d[:, RR:H, :])
              nc.sync.dma_start(x2[CIN:128, RR:H, :], x_d[:, RR:H, :])
              wdma(wspl[:, NTAP : NKT * NTAP, :], wspl_d[:, NTAP : NKT * NTAP, :])

              rhsW = cpool.tile([128, H, WB], bf16, tag="rhsW")

              def emit_chunk(r0, r1, t0=0, t1=NKT):
                  rows = r1 - r0
                  wb = (t1 - t0) * W
                  sl = (slice(None), slice(r0, r1), slice(t0 * W, t1 * W))
                  v = spool.tile([128, rows, wb], f32, tag="v")
                  for t in range(t0, t1):
                      nc.scalar.activation(
                          v[:, :, (t - t0) * W : (t - t0 + 1) * W],
                          x2[:, r0:r1, :],
                          Act.Abs,
                          bias=bneg[:, t : t + 1],
                          scale=2.5,
                      )
                  m = spool.tile([128, rows, wb], ew, tag="m")
                  nc.vector.tensor_scalar(m[:], v[:], 2.0, 2.0, Alu.min, Alu.subtract)
                  n = spool.tile([128, rows, wb], ew, tag="n")
                  nc.vector.tensor_scalar(n[:], v[:], 1.0, 1.0, Alu.min, Alu.subtract)
                  m2 = spool.tile([128, rows, wb], ew, tag="m2")
                  n2q = spool.tile([128, rows, wb], ew, tag="n2q")
                  if cfg["sq_engine"] == "act":
                      nc.scalar.activation(m2[:], m[:], Act.Square)
                      nc.scalar.activation(n2q[:], n[:], Act.Square, scale=2.0)
                  elif cfg["sq_engine"] == "dve":
                      nc.vector.tensor_tensor(m2[:], m[:], m[:], Alu.mult)
                      nc.vector.scalar_tensor_tensor(
                          n2q[:], n[:], 4.0, n[:], Alu.mult, Alu.mult
                      )
                  else:  # split
                      nc.scalar.activation(m2[:], m[:], Act.Square)
                      nc.vector.scalar_tensor_tensor(
                          n2q[:], n[:], 4.0, n[:], Alu.mult, Alu.mult
                      )
                  m3 = spool.tile([128, rows, wb], ew, tag="m3")
                  nc.vector.tensor_tensor(m3[:], m2[:], m[:], Alu.mult)
                  n3q = spool.tile([128, rows, wb], ew, tag="n3q")
                  nc.vector.tensor_tensor(n3q[:], n2q[:], n[:], Alu.mult)
                  nc.vector.tensor_tensor(rhsW[sl], n3q[:], m3[:], Alu.subtract)

              chunks = _chunks_for(cfg["nchunks"])

              emit_silu(RS, H)
              ps_banks = [
                  ppool.tile([128, NFREE], f32, tag="ps", name=f"ps{g}")
                  for g in range(NGRP)
              ]
              if cfg["do_base"] and cfg["shift_base"]:
                  for grp in range(NGRP):
                      first = True
                      for j in (0, 2, 3):
                          tapA = 2 * j
                          khA, kwA = divmod(tapA, K)
                          rv = silu2[
                              :, 9 * grp + khA : 9 * grp + khA + GROW, kwA : kwA + WO
                          ]
                          nc.tensor.matmul(
                              ps_banks[grp][:], wbase[:, j, :], rv,
                              start=first, stop=False,
                          )
                          first = False
                      sv = silu2[
                          0:CIN, 9 * grp + 2 : 9 * grp + 2 + GROW, 2 : 2 + WO
                      ]
                      nc.tensor.matmul(
                          ps_banks[grp][:], wbase[0:CIN, 4, :], sv,
                          start=False, stop=False,
                      )

              for grp in range(NGRP):
                  for (r0, r1, g0) in chunks:
                      if g0 == grp:
                          if r0 > 0:
                              emit_silu(r0, r1)
                          if cfg["do_ew"]:
                              if grp == 0 and r0 == 0:
                                  emit_chunk(r0, r1, 0, 2)
                                  emit_chunk(r0, r1, 2, NKT)
                              else:
                                  emit_chunk(r0, r1)
                          if cfg["shift_base"]:
                              emit_siluB(r0, r1)

                  ps = ps_banks[grp]
                  first = not (cfg["do_base"] and cfg["shift_base"])
                  if cfg["do_base"] and not cfg["shift_base"]:
                      for tap in range(NTAP):
                          kh, kw = divmod(tap, K)
                          sv = silu2[
                              0:CIN, 9 * grp + kh : 9 * grp + kh + GROW, kw : kw + WO
                          ]
                          nc.tensor.matmul(
                              ps[:],
                              wbase_flat[0:CIN, tap, :],
                              sv,
                              start=first,
                              stop=False,
                          )
                          first = False

                  nsp = NTAP * NKT
                  k = 0
                  for t in range(NKT if cfg["do_spline"] else 0):
                      for tap in range(NTAP):
                          kh, kw = divmod(tap, K)
                          rv = rhsW[
                              :,
                              9 * grp + kh : 9 * grp + kh + GROW,
                              t * W + kw : t * W + kw + WO,
                          ]
                          nc.tensor.matmul(
                              ps[:],
                              wspl[:, t * NTAP + tap, :],
                              rv,
                              start=first,
                              stop=(k == nsp - 1)
                              and not (cfg["do_base"] and cfg["shift_base"]),
                          )
                          first = False
                          k += 1
                  if cfg["do_base"] and cfg["shift_base"]:
                      rv = siluB[:, 9 * grp : 9 * grp + GROW, 2 : 2 + WO]
                      nc.tensor.matmul(
                          ps[:], wbase[:, 1, :], rv, start=False, stop=True
                      )

                  ev = spool.tile([128, NFREE], f32, tag="ev")
                  if cfg["evict_engine"] == "act":
                      nc.scalar.copy(ev[:], ps[:])
                  else:
                      nc.vector.tensor_copy(ev[:], ps[:])
                  nc.sync.dma_start(y_d[:, GROW * grp : GROW * (grp + 1), :], ev[:])

    _split_excess_waits(nc)
    _CACHE[key] = nc
    return nc


def _prep_weights(base_weight, spline_weight, spline_scaler):
    """Fold scaler and 1/6 into spline weights; lay out matmul lhsT tiles."""
    sw = (spline_weight * spline_scaler[:, :, None]).astype(np.float32) / 6.0
    # sw: [COUT, 576, 8]; feature index i = c*9 + tap
    sw4 = sw.reshape(COUT, CIN, NTAP, 8)  # [co, c, tap, g]
    # wspl[p, tap*4+t, co] = sw4[co, c, tap, 2t+gh], p = gh*64 + c
    w = np.transpose(sw4, (1, 2, 3, 0))  # [c, tap, g, co]
    w = w.reshape(CIN, NTAP, NKT, 2, COUT)  # g = 2t + gh -> [c, tap, t, gh, co]
    w = np.transpose(w, (3, 2, 0, 1, 4))  # [gh, t, c, tap, co]
    w = w.reshape(2, NKT, CIN, NTAP, COUT)
    w = np.transpose(w, (0, 2, 1, 3, 4))  # [gh, c, t, tap, co]
    wspl = w.reshape(2 * CIN, NKT * NTAP, COUT).astype(ml_dtypes.bfloat16)

    wb = base_weight.reshape(COUT, CIN, NTAP)  # [co, c, tap]
    wb_ct = np.transpose(wb, (1, 2, 0))  # [c, tap, co]
    wbase = np.zeros((128, 5, COUT), np.float32)
    for j in range(5):
        wbase[0:CIN, j, :] = wb_ct[:, 2 * j, :]
        if j < 4:
            wbase[CIN:128, j, :] = wb_ct[:, 2 * j + 1, :]
    wbase = wbase.astype(ml_dtypes.bfloat16)
    wbflat = np.ascontiguousarray(wb_ct).astype(ml_dtypes.bfloat16)

    gh = np.arange(128) // CIN  # 0 for p<64, 1 otherwise
    t = np.arange(NKT)
    bneg = (3.5 - (2 * t[None, :] + gh[:, None])).astype(np.float32)  # [128, 4]
    return wspl, wbase, wbflat, bneg


def _in_maps(x, base_weight, spline_weight, spline_scaler):
    wspl, wbase, wbflat, bneg = _prep_weights(base_weight, spline_weight, spline_scaler)
    return [
        {
            "x": np.ascontiguousarray(x[b]).astype(np.float32),
            "wspl": wspl,
            "wbase2": wbase,
            "wbflat": wbflat,
            "betaneg": bneg,
        }
        for b in range(B)
    ]


def kernel(x, base_weight, spline_weight, spline_scaler):
    from concourse.bass_utils import run_bass_kernel_spmd

    nc = _build()
    in_maps = _in_maps(x, base_weight, spline_weight, spline_scaler)
    res = run_bass_kernel_spmd(nc, in_maps, core_ids=list(range(B)))
    out = np.stack([res.results[b]["y"] for b in range(B)])  # [8, 128, 54, 54]
    return out.astype(np.float32)



# revision 7
# speedup vs baseline: 1.7521x; 1.7521x over previous
"""Trainium2 Bass kernel for CKANConv2d (KAN conv: SiLU base + B-spline path).

Math: for each output pixel p and output channel co:
  out[co,p] = sum_{c,kh,kw} silu(x[c,p+k]) * Wb[co,(c,kh,kw)]
            + sum_{c,kh,kw,g} B_g(x[c,p+k]) * Ws[co,(c,kh,kw),g]
with B_g the order-3 uniform B-spline bases over knots {-2.2 + 0.4j}.

On-chip approximation (v = |2.5x - c_g|, clamped to [0,2]):
  6*B_g(x) = 4n^3 - m^3  ~=  4a(1 - s^2)^2,  s = sin((pi/4)*v)
(max abs err ~1% of basis scale at a=0.99 — below the fp8 quantization
noise of the spline path). Because s^2 is even in (x - c'), no abs is
needed: y = clamp(x, c'+-0.8) in ONE tensor_scalar, s = Sin(1.9635*y +
bias_p) stays inside the table-exact [-pi/2, pi/2] argument range, and
the basis is (2*sqrt(a)*(1 - s^2))^2 via one square + one affine-square.

Bases form a [128, 4, 56, 56] fp8e4 field: plane tb covers g = 2*tb+gh
(gh = partition//64, x duplicated across halves). Spline matmuls run in
fp8e4 MatmulPerfMode.DoubleRow: one matmul contracts 2 planes (256
rows) at 0.5 cycles/out-row — 4x the bf16 rate. Both weight sets are
pre-scaled by S=256 so fp8 weights sit in e4m3's normal range; the
PSUM eviction multiplies by 1/S. The base path stays bf16 (it carries
~87% of output variance; the fp8 spline path carries ~13%).

Sharding: data-parallel over batch, 1 image per NeuronCore (8 cores).
"""
import numpy as np
import ml_dtypes

B, CIN, H, W = 8, 64, 56, 56
COUT, K = 128, 3
HO = WO = 54
NTAP = K * K  # 9
NTB = 4  # basis planes: plane tb covers g = 2*tb + gh
NGRP = 6  # output row groups of 9 rows each
GROW = 9  # output rows per group
NFREE = GROW * WO  # 486 <= 512 (one PSUM bank)
S = 256.0  # weight pre-scale (fp8 range); evict multiplies by 1/S
ALPHA = 0.99  # amplitude of the sin^2 basis approximation
SCL = 2.5 * np.pi / 4  # x-units -> (pi/4)*v units

_CACHE = {}


def _patch_tile_tail_drain():
    """walrus in this env rejects the Tile tail Drain when it carries >1
    sync waits; split them into a chain of single-wait Drains."""
    import concourse.tile as tile
    from concourse.vector_clock import ScopedClock

    if getattr(tile.TileContext, "_drain_patched", False):
        return

    def _patched(self, tick_clock, wait_clock):
        drain_inst = self.nc.sync.drain()
        wait_clock.add_sem_waits(
            drain_inst.ins, ScopedClock({None: tick_clock.global_clock})
        )
        si = drain_inst.ins.sync_info
        waits = list(si.on_wait) if si is not None else []
        if len(waits) > 1:
            si.on_wait = waits[:1]
            handles = {h.num: h for h in self.sems.allocated().values()}
            for w in waits[1:]:
                extra = self.nc.sync.drain()
                extra.wait_op(handles[w.id], w.wait_value, "sem-ge")
        self.nc.all_engine_barrier()
        assert self.sems is not None
        popped = self.nc._tile_sem_poison_stack.pop()
        assert popped is self._sem_poison
        self.nc.clear_and_free_semaphores(list(self.sems.allocated().values()))
        self.nc.all_engine_barrier()

    tile.TileContext._drain_and_barrier = _patched
    tile.TileContext._drain_patched = True


def _split_excess_waits(nc, max_waits=1):
    """This walrus build encodes at most one sync-wait per instruction.
    Move extra waits onto same-engine NoOps inserted just before."""
    import bass_rust
    from concourse import mybir

    for f in nc.m.functions:
        for bb in f.blocks:
            new = []
            for ins in bb.instructions:
                si = ins.sync_info
                if si is not None and len(si.on_wait) > max_waits:
                    waits = list(si.on_wait)
                    for w in waits[: len(waits) - max_waits]:
                        nop = mybir.InstNoOp(
                            name=nc.get_next_instruction_name(), ins=[], outs=[]
                        )
                        nop.engine = ins.engine
                        h = bass_rust.SemaphoreHandle(name=w.ant_name, num=w.id)
                        bass_rust.wait_op(nop, h, w.wait_value, "sem-ge", False)
                        nc.register_instruction(nop, overwrite=True)
                        new.append(nop)
                    si.on_wait = waits[len(waits) - max_waits :]
                new.append(ins)
            bb.instructions = new


CFG = {
    "nchunks": 6,
    "mid_act": 1,   # s^2 planes computed on ACT (rest: one DVE TT)
    "fin_act": 0,   # final-square planes on ACT (rest: DVE affine+TT+cast DMA)
    "evict_engine": "dve",
    "wdma": "gpsimd",
    "sbufs": 3,
    "pbufs": 6,
    "lookahead": 1,
}


def _chunks_for(nchunks):
    """Partition input rows 0..55 into nchunks contiguous chunks such that
    matmul group r (needs input rows 9r..9r+10) only depends on chunks
    emitted at or before group r. Returns list of (r0, r1, first_group)."""
    gper = [len(x) for x in np.array_split(np.arange(NGRP), nchunks)]
    out = []
    g0 = 0
    r_prev = 0
    for ng in gper:
        glast = g0 + ng - 1
        r1 = min(9 * glast + 11, H)
        out.append((r_prev, r1, g0))
        r_prev = r1
        g0 += ng
    return out


def _build(cfg=None):
    key = ("nc", tuple(sorted((cfg or CFG).items())))
    if key in _CACHE:
        return _CACHE[key]
    cfg = dict(CFG, **(cfg or {}))
    _patch_tile_tail_drain()
    import concourse.bass as bass
    import concourse.tile as tile
    from concourse import mybir

    f32 = mybir.dt.float32
    bf16 = mybir.dt.bfloat16
    fp8 = mybir.dt.float8e4
    Alu = mybir.AluOpType
    Act = mybir.ActivationFunctionType
    PM = mybir.MatmulPerfMode

    AF = float(2.0 * np.sqrt(ALPHA))

    nc = bass.Bass("TRN2", num_swdge_queues=4)
    x_d = nc.dram_tensor("x", [CIN, H, W], f32, kind="ExternalInput").ap()
    wspl_d = nc.dram_tensor(
        "wspl8", [128, NTAP * 2, 2, 128], fp8, kind="ExternalInput"
    ).ap()
    wbase_d = nc.dram_tensor("wbase2", [128, 5, 128], bf16, kind="ExternalInput").ap()
    # ecoef cols: [0:4)=clamp lo, [4:8)=clamp hi, [8:12)=sin bias, 12=AF
    ecoef_d = nc.dram_tensor("ecoef", [128, 13], f32, kind="ExternalInput").ap()
    y_d = nc.dram_tensor("y", [128, HO, WO], f32, kind="ExternalOutput").ap()

    nrep = cfg.get("replicate", 1)
    with tile.TileContext(nc) as tc:
        with (
            tc.tile_pool(name="consts", bufs=1) as cpool,
            tc.tile_pool(name="scratch", bufs=cfg.get("sbufs", 2)) as spool,
            tc.tile_pool(name="psum", bufs=cfg.get("pbufs", 6), space="PSUM") as ppool,
        ):
          for _rep in range(nrep):
            ecoef = cpool.tile([128, 13], f32, tag="ecoef")
            nc.sync.dma_start(ecoef[:], ecoef_d)
            # x first, row-chunked so chunk-0 elementwise starts immediately
            RR = 12  # first x-DMA rows (chunk 0 + 1 shifted-silu row)
            x2 = cpool.tile([128, H, W], f32, tag="x2")
            nc.sync.dma_start(x2[0:CIN, 0:RR, :], x_d[:, 0:RR, :])
            nc.sync.dma_start(x2[CIN:128, 0:RR, :], x_d[:, 0:RR, :])
            wdma = nc.gpsimd.dma_start if cfg["wdma"] == "gpsimd" else nc.sync.dma_start
            wbase = cpool.tile([128, 5, 128], bf16, tag="wbase")
            wdma(wbase[:], wbase_d)
            wspl = cpool.tile([128, NTAP * 2, 2, 128], fp8, tag="wspl")
            wdma(wspl[:], wspl_d)

            RS = 11  # silu/chunk-0 row boundary
            silu2 = cpool.tile([128, H, W], bf16, tag="silu2")
            siluB = cpool.tile([128, H, W], bf16, tag="siluB")
            xb = cpool.tile([128, H, W], bf16, tag="xb")

            def emit_silu(r0, r1):
                # lower = silu(x) on ACT; shifted upper halves via SBUF-SBUF
                # DMA partition-offset copies (DMA engines are idle here)
                nc.scalar.activation(
                    silu2[0:CIN, r0:r1, :], x2[0:CIN, r0:r1, :], Act.Silu
                )
                # S_A upper: shift (0,+1)
                nc.sync.dma_start(
                    silu2[CIN:128, r0:r1, 0 : W - 1],
                    silu2[0:CIN, r0:r1, 1:W],
                )

            def emit_siluB(r0, r1):
                # S_B: lower = silu(x); upper = silu(x) shifted (+1,-2)
                r1u = min(r1, H - 1)
                nc.sync.dma_start(
                    siluB[CIN:128, r0:r1u, 2:W],
                    silu2[0:CIN, r0 + 1 : r1u + 1, 0 : W - 2],
                )
                nc.sync.dma_start(
                    siluB[0:CIN, r0:r1, :], silu2[0:CIN, r0:r1, :]
                )

            emit_silu(0, RS)
            nc.sync.dma_start(x2[0:CIN, RR:H, :], x_d[:, RR:H, :])
            nc.sync.dma_start(x2[CIN:128, RR:H, :], x_d[:, RR:H, :])

            rhs8 = cpool.tile([128, NTB, H, W], fp8, tag="rhs8")

            def emit_chunk(r0, r1, t0=0, t1=NTB):
                rows = r1 - r0
                ntb = t1 - t0
                nc.vector.tensor_scalar(
                    xb[:, r0:r1, :], x2[:, r0:r1, :], 0.0, None, Alu.bypass
                )
                yc = spool.tile([128, ntb, rows, W], bf16, tag=f"y{t0}")
                for t in range(t0, t1):
                    nc.vector.tensor_scalar(
                        yc[:, t - t0], xb[:, r0:r1, :],
                        ecoef[:, t : t + 1], ecoef[:, 4 + t : 5 + t],
                        Alu.max, Alu.min,
                    )
                sc_ = spool.tile([128, ntb, rows, W], bf16, tag=f"s{t0}")
                for t in range(t0, t1):
                    nc.scalar.activation(
                        sc_[:, t - t0], yc[:, t - t0], Act.Sin,
                        bias=ecoef[:, 8 + t : 9 + t], scale=float(SCL),
                    )
                s2 = spool.tile([128, ntb, rows, W], bf16, tag=f"s2{t0}")
                ma = min(cfg["mid_act"], ntb)
                for t in range(ma):
                    nc.scalar.activation(s2[:, t], sc_[:, t], Act.Square)
                if ma < ntb:
                    nc.vector.tensor_tensor(
                        s2[:, ma:ntb], sc_[:, ma:ntb], sc_[:, ma:ntb], Alu.mult
                    )
                # final: q = (AF*(1 - s2))^2 -> fp8
                fa = min(cfg["fin_act"], ntb)
                for t in range(fa):
                    nc.scalar.activation(
                        rhs8[:, t0 + t, r0:r1, :], s2[:, t], Act.Square,
                        bias=ecoef[:, 12:13], scale=-AF,
                    )
                if fa < ntb:
                    t1a = spool.tile([128, ntb - fa, rows, W], bf16, tag=f"t1{t0}")
                    nc.vector.tensor_scalar(
                        t1a[:], s2[:, fa:ntb], -AF, AF, Alu.mult, Alu.add
                    )
                    rb = spool.tile([128, ntb - fa, rows, W], bf16, tag=f"rb{t0}")
                    nc.vector.tensor_tensor(rb[:], t1a[:], t1a[:], Alu.mult)
                    nc.gpsimd.dma_start(rhs8[:, t0 + fa : t1, r0:r1, :], rb[:])

            chunks = _chunks_for(cfg["nchunks"])

            emit_silu(RS, H)
            ps_banks = [
                ppool.tile([128, NFREE], f32, tag="ps", name=f"ps{g}")
                for g in range(NGRP)
            ]
            # base path: 4 paired taps per group, first matmuls of each bank
            for grp in range(NGRP):
                first = True
                for j in (0, 2, 3):
                    tapA = 2 * j
                    khA, kwA = divmod(tapA, K)
                    rv = silu2[
                        :, 9 * grp + khA : 9 * grp + khA + GROW, kwA : kwA + WO
                    ]
                    nc.tensor.matmul(
                        ps_banks[grp][:], wbase[:, j, :], rv,
                        start=first, stop=False,
                    )
                    first = False
                sv = silu2[0:CIN, 9 * grp + 2 : 9 * grp + 2 + GROW, 2 : 2 + WO]
                nc.tensor.matmul(
                    ps_banks[grp][:], wbase[0:CIN, 4, :], sv,
                    start=False, stop=False,
                )

            LA = cfg.get("lookahead", 0)
            emitted = set()

            def emit_chunks_upto(gmax):
                for ci, (r0, r1, g0) in enumerate(chunks):
                    if ci in emitted or g0 > gmax:
                        continue
                    emitted.add(ci)
                    if r0 > 0:
                        emit_silu(r0, r1)
                    if r0 == 0:
                        emit_chunk(r0, r1, 0, 2)
                        emit_chunk(r0, r1, 2, NTB)
                    else:
                        emit_chunk(r0, r1)
                    emit_siluB(r0, r1)

            for grp in range(NGRP):
                emit_chunks_upto(grp + LA)

                ps = ps_banks[grp]
                for a in range(2):
                    for tap in range(NTAP):
                        kh, kw = divmod(tap, K)
                        rv = rhs8[
                            :, 2 * a : 2 * a + 2,
                            9 * grp + kh : 9 * grp + kh + GROW,
                            kw : kw + WO,
                        ]
                        nc.tensor.matmul(
                            ps[:], wspl[:, tap * 2 + a], rv,
                            start=False, stop=False, perf_mode=PM.DoubleRow,
                        )
                # base tail: S_B pair closes the accumulation group
                rv = siluB[:, 9 * grp : 9 * grp + GROW, 2 : 2 + WO]
                nc.tensor.matmul(ps[:], wbase[:, 1, :], rv, start=False, stop=True)

                ev = spool.tile([128, NFREE], f32, tag="ev")
                if cfg["evict_engine"] == "act":
                    nc.scalar.activation(ev[:], ps[:], Act.Identity, scale=1.0 / S)
                else:
                    nc.vector.tensor_scalar(ev[:], ps[:], 1.0 / S, None, Alu.mult)
                nc.sync.dma_start(y_d[:, GROW * grp : GROW * (grp + 1), :], ev[:])

    _split_excess_waits(nc)
    _CACHE[key] = nc
    return nc


def _prep_weights(base_weight, spline_weight, spline_scaler):
    """Fold scaler, 1/6 and the S pre-scale into the weights; lay out lhsT."""
    sw = (spline_weight * spline_scaler[:, :, None]).astype(np.float32) * (S / 6.0)
    # sw: [COUT, 576, 8]; feature index i = c*9 + tap
    sw4 = sw.reshape(COUT, CIN, NTAP, 8)  # [co, c, tap, g]
    w = np.transpose(sw4, (1, 2, 3, 0))  # [c, tap, g, co]
    # g = 4a + 2j + gh
    w = w.reshape(CIN, NTAP, 2, 2, 2, COUT)  # [c, tap, a, j, gh, co]
    w = np.transpose(w, (4, 0, 1, 2, 3, 5))  # [gh, c, tap, a, j, co]
    wspl8 = np.ascontiguousarray(w).reshape(128, NTAP * 2, 2, COUT).astype(
        ml_dtypes.float8_e4m3
    )

    wb = base_weight.reshape(COUT, CIN, NTAP) * S  # [co, c, tap]
    wb_ct = np.transpose(wb, (1, 2, 0))  # [c, tap, co]
    wbase = np.zeros((128, 5, COUT), np.float32)
    for j in range(5):
        wbase[0:CIN, j, :] = wb_ct[:, 2 * j, :]
        if j < 4:
            wbase[CIN:128, j, :] = wb_ct[:, 2 * j + 1, :]
    wbase = wbase.astype(ml_dtypes.bfloat16)

    gh = np.arange(128) // CIN  # 0 for p<64, 1 otherwise
    t = np.arange(NTB)
    cp = ((2 * t[None, :] + gh[:, None]) - 3.5) / 2.5  # [128, 4] centers, x units
    ecoef = np.zeros((128, 13), np.float32)
    ecoef[:, 0:4] = cp - 0.8
    ecoef[:, 4:8] = cp + 0.8
    ecoef[:, 8:12] = -SCL * cp
    ecoef[:, 12] = 2.0 * np.sqrt(ALPHA)
    return wspl8, wbase, ecoef.astype(np.float32)


def _in_maps(x, base_weight, spline_weight, spline_scaler):
    wspl8, wbase, ecoef = _prep_weights(base_weight, spline_weight, spline_scaler)
    return [
        {
            "x": np.ascontiguousarray(x[b]).astype(np.float32),
            "wspl8": wspl8,
            "wbase2": wbase,
            "ecoef": ecoef,
        }
        for b in range(B)
    ]


def kernel(x, base_weight, spline_weight, spline_scaler):
    from concourse.bass_utils import run_bass_kernel_spmd

    nc = _build()
    in_maps = _in_maps(x, base_weight, spline_weight, spline_scaler)
    res = run_bass_kernel_spmd(nc, in_maps, core_ids=list(range(B)))
    out = np.stack([res.results[b]["y"] for b in range(B)])  # [8, 128, 54, 54]
    return out.astype(np.float32)
